# revision 4
# baseline (speedup 1.0000x reference)
"""Fused multi-head attention kernel for Trainium2 (8 NeuronCores, SPMD).

Problem: B=512, T=128, C=768, H=12, D=64 causal MHA:
    qkv = x @ w_qkv.T ; per-head causal softmax(q k^T / 8) @ v ; proj + bias.

Sharding: data-parallel over batch, 64 batches per core. Host-side prep is
layout only (transposes); all FLOPs run on device.

Per-core dataflow per batch (T=128 tokens on 128 partitions):
  - qk^T  [12*128, t] chunks via f32r matmuls (w stationary, 4-batch groups
    so the moving dim is 512), cast to bf16 on PSUM evacuation.
  - v     [t, 768] natural layout via f32r matmuls (xT chunk stationary).
  - per head: S = qT_h^T @ kT_h (bf16), mask add (DVE), exp*0.125 with
    fused row-sum (ACT accum_out), reciprocal, scale+bf16 cast, PE
    transpose of P, O^T = v_h^T... i.e. matmul(lhsT=v_h, rhs=P^T) giving
    proj-ready [c, t] chunks (head pairs share one PSUM tile via column
    tiling).
  - proj: f32r matmuls (OT chunk stationary), bias added during PSUM
    evacuation, DMA out in natural [t, C] layout.
"""

import numpy as np

import concourse.bass as bass
import concourse.tile as tile
from concourse import bacc, mybir
from concourse.bass_utils import run_bass_kernel_spmd
from concourse.masks import make_identity

F32 = mybir.dt.float32
F32R = mybir.dt.float32r
BF16 = mybir.dt.bfloat16

N_CORES = 8
B_TOTAL = 512
T = 128
C = 768
H = 12
D = 64
KC = C // 128  # 6 contraction chunks
B_CORE = B_TOTAL // N_CORES  # 64
GB = 4  # batches per group (moving dim 4*128=512)
NEG = -1.0e9


def _build(b_core=B_CORE, att_bf16=True):
    nc = bacc.Bacc()
    xT_h = nc.dram_tensor("xT", [b_core, KC, 128, T], F32R, kind="ExternalInput")
    wqkvT_h = nc.dram_tensor("wqkvT", [C, 3 * C], F32R, kind="ExternalInput")
    wpT_h = nc.dram_tensor("wpT", [C, C], F32R, kind="ExternalInput")
    bias_h = nc.dram_tensor("bias", [C], F32, kind="ExternalInput")
    y_h = nc.dram_tensor("y", [b_core, T, C], F32, kind="ExternalOutput")

    att_dt = BF16 if att_bf16 else F32
    n_groups = b_core // GB

    with tile.TileContext(nc) as tc:
        with (
            tc.tile_pool(name="const", bufs=1) as constp,
            tc.tile_pool(name="xt", bufs=2) as xtp,
            tc.tile_pool(name="qkt", bufs=2) as qktp,
            tc.tile_pool(name="vsb", bufs=2) as vp,
            tc.tile_pool(name="ot", bufs=2) as otp,
            tc.tile_pool(name="ysb", bufs=2) as yp,
            tc.tile_pool(name="small", bufs=2) as smallp,
            tc.tile_pool(name="stats", bufs=3) as statsp,
            tc.tile_pool(name="qkps", bufs=2, space="PSUM") as qkpsp,
            tc.tile_pool(name="attps", bufs=4, space="PSUM") as attpsp,
            tc.tile_pool(name="pjps", bufs=2, space="PSUM") as pjpsp,
        ):
            # ---- constants / weights (loaded once) ----
            wqkv = constp.tile([128, KC, 3 * C], F32R, tag="wqkv")
            nc.sync.dma_start(
                out=wqkv[:], in_=wqkvT_h[:].rearrange("(k p) o -> p k o", p=128)
            )
            wp = constp.tile([128, KC, C], F32R, tag="wp")
            nc.sync.dma_start(
                out=wp[:], in_=wpT_h[:].rearrange("(k p) o -> p k o", p=128)
            )
            bias_bc = constp.tile([128, C], F32, tag="bias")
            b_src = bias_h[:]
            b_bcast = bass.AP(
                tensor=b_src.tensor, offset=b_src.offset, ap=[[0, 128]] + list(b_src.ap)
            )
            nc.gpsimd.dma_start(out=bias_bc[:], in_=b_bcast)

            mask = constp.tile([128, T], F32, tag="mask")
            nc.gpsimd.memset(mask[:], 0.0)
            # keep (in_=0) where t - s >= 0, else fill NEG  (causal)
            nc.gpsimd.affine_select(
                out=mask[:],
                in_=mask[:],
                compare_op=mybir.AluOpType.is_ge,
                fill=NEG,
                base=0,
                pattern=[[-1, T]],
                channel_multiplier=1,
            )
            ident = constp.tile([128, 128], att_dt, tag="ident")
            make_identity(nc, ident[:])

            for g in range(n_groups):
                # ---- load 4 batches of xT ----
                xt = xtp.tile([128, KC, GB, T], F32R, tag="xt")
                for bi in range(GB):
                    b = g * GB + bi
                    nc.sync.dma_start(
                        out=xt[:, :, bi, :],
                        in_=xT_h[b].rearrange("k p t -> p k t"),
                    )

                # ---- q^T, k^T chunks for the whole group ----
                qkt = qktp.tile([128, 2 * KC, GB, T], att_dt, tag="qkt")
                for r in range(2 * KC):
                    ps = qkpsp.tile([128, GB, T], F32, tag="qkps")
                    for kc in range(KC):
                        nc.tensor.matmul(
                            ps[:],
                            lhsT=wqkv[:, kc, 128 * r : 128 * r + 128],
                            rhs=xt[:, kc, :, :],
                            start=(kc == 0),
                            stop=(kc == KC - 1),
                        )
                    eng = nc.vector if (r % 2 == 0) else nc.scalar
                    if eng is nc.vector:
                        eng.tensor_copy(qkt[:, r], ps[:])
                    else:
                        eng.copy(qkt[:, r], ps[:])

                for bi in range(GB):
                    b = g * GB + bi
                    # ---- v in natural [t, C] layout ----
                    vsb = vp.tile([128, C], att_dt, tag="vsb")
                    for half in range(2):
                        vps = qkpsp.tile([128, 384], F32, tag="qkps")
                        for kc in range(KC):
                            nc.tensor.matmul(
                                vps[:],
                                lhsT=xt[:, kc, bi, :],
                                rhs=wqkv[
                                    :, kc, 2 * C + 384 * half : 2 * C + 384 * (half + 1)
                                ],
                                start=(kc == 0),
                                stop=(kc == KC - 1),
                            )
                        nc.scalar.copy(vsb[:, 384 * half : 384 * (half + 1)], vps[:])

                    # ---- attention, head pairs ----
                    ot = otp.tile([128, KC, T], F32R, tag="ot")
                    for hp in range(KC):
                        ops = attpsp.tile([128, T], F32, tag="attps")
                        for hh in range(2):
                            h = 2 * hp + hh
                            po = 64 * (h % 2)
                            ch = h // 2
                            sps = attpsp.tile([128, T], F32, tag="attps")
                            nc.tensor.matmul(
                                sps[:],
                                lhsT=qkt[po : po + 64, ch, bi, :],
                                rhs=qkt[po : po + 64, KC + ch, bi, :],
                                start=True,
                                stop=True,
                            )
                            sm = smallp.tile([128, T], F32, tag="sm")
                            nc.vector.tensor_add(sm[:], sps[:], mask[:])
                            p1 = smallp.tile([128, T], F32, tag="p1")
                            sums = statsp.tile([128, 1], F32, tag="sums")
                            nc.scalar.activation(
                                out=p1[:],
                                in_=sm[:],
                                func=mybir.ActivationFunctionType.Exp,
                                scale=0.125,
                                accum_out=sums[:],
                            )
                            recip = statsp.tile([128, 1], F32, tag="recip")
                            nc.vector.reciprocal(recip[:], sums[:])
                            p2 = smallp.tile([128, T], att_dt, tag="p2")
                            nc.scalar.activation(
                                out=p2[:],
                                in_=p1[:],
                                func=mybir.ActivationFunctionType.Copy,
                                scale=recip[:],
                            )
                            pts = attpsp.tile([128, T], att_dt, tag="attps")
                            nc.tensor.transpose(pts[:], p2[:], ident[:])
                            ptsb = smallp.tile([128, T], att_dt, tag="ptsb")
                            nc.vector.tensor_copy(ptsb[:], pts[:])
                            nc.tensor.matmul(
                                ops[64 * hh : 64 * hh + 64, :],
                                lhsT=vsb[:, 64 * h : 64 * h + 64],
                                rhs=ptsb[:],
                                start=True,
                                stop=True,
                                tile_position=(0, 64 * hh),
                            )
                        nc.scalar.copy(ot[:, hp, :], ops[:])

                    # ---- proj + bias ----
                    ysb = yp.tile([128, C], F32, tag="ysb")
                    for half in range(2):
                        pps = pjpsp.tile([128, 384], F32, tag="pjps")
                        for kc in range(KC):
                            nc.tensor.matmul(
                                pps[:],
                                lhsT=ot[:, kc, :],
                                rhs=wp[:, kc, 384 * half : 384 * (half + 1)],
                                start=(kc == 0),
                                stop=(kc == KC - 1),
                            )
                        nc.vector.tensor_add(
                            ysb[:, 384 * half : 384 * (half + 1)],
                            pps[:],
                            bias_bc[:, 384 * half : 384 * (half + 1)],
                        )
                    nc.sync.dma_start(out=y_h[b], in_=ysb[:])

    nc.finalize()
    return nc


_NC_CACHE = {}


def _get_nc(b_core=B_CORE, att_bf16=True):
    key = (b_core, att_bf16)
    if key not in _NC_CACHE:
        _NC_CACHE[key] = _build(b_core, att_bf16)
    return _NC_CACHE[key]


def _prep_inputs(x, w_qkv, w_proj, b_proj, b_core):
    x = np.asarray(x, dtype=np.float32)
    n_cores = x.shape[0] // b_core
    # [B, T, C] -> [B, C, T] -> [B, KC, 128, T]
    xT = np.ascontiguousarray(x.transpose(0, 2, 1)).reshape(x.shape[0], KC, 128, T)
    wqkvT = np.ascontiguousarray(np.asarray(w_qkv, dtype=np.float32).T)
    wpT = np.ascontiguousarray(np.asarray(w_proj, dtype=np.float32).T)
    bias = np.ascontiguousarray(np.asarray(b_proj, dtype=np.float32))
    in_maps = []
    for c in range(n_cores):
        in_maps.append(
            {
                "xT": np.ascontiguousarray(xT[c * b_core : (c + 1) * b_core]),
                "wqkvT": wqkvT,
                "wpT": wpT,
                "bias": bias,
            }
        )
    return in_maps


def run(x, w_qkv, w_proj, b_proj, b_core=B_CORE, att_bf16=True, trace=False):
    nc = _get_nc(b_core, att_bf16)
    n_cores = x.shape[0] // b_core
    in_maps = _prep_inputs(x, w_qkv, w_proj, b_proj, b_core)
    res = run_bass_kernel_spmd(nc, in_maps, list(range(n_cores)), trace=trace)
    y = np.concatenate([res.results[i]["y"] for i in range(n_cores)], axis=0)
    return y, res


def kernel(x, w_qkv, w_proj, b_proj):
    y, _ = run(x, w_qkv, w_proj, b_proj)
    return y
